# revision 5
# baseline (speedup 1.0000x reference)
"""Trainium2 kernel for nn_MyModel_87522843560950.

Reference computes, per replicate k (row of a (64, 500000) f32 array):
  x_0 = 0;  x_{t+1} = x_t - 0.1 * mean(2*(x_t - data_k))  for 100 iters.
Algebraically x_{t+1} = 0.8*x_t + 0.2*mean(data_k), so
  x_100 = mean(data_k) * (1 - 0.8**100).
(1 - 0.8**100) differs from 1 by ~2e-10 — far below f32 resolution — so the
whole problem is a row-mean over the (64, 500000) array: memory-bound.

Sharding: data-parallel over the replicate axis. Core c takes rows
[8c, 8c+8), viewed as (128, 31250) f32 (each row spans 16 partitions,
31250 contiguous elements per partition). On-device per core:
  - tiled DMA loads (HBM -> SBUF), striped across SWDGE queues
  - vector reduce_sum per tile -> partials (128, n_tiles)
  - final reduce -> per-partition sums (128, 1)
  - matmul with a 0/1 selection matrix (128x8, loaded as a tiny second
    input) sums each group of 16 partitions -> (8, 1) PSUM
  - scale by (1 - 0.8**100)/500000 on ScalarE, DMA out (8,)
Gather: concatenate the 8 per-core (8,) outputs -> (64,).
"""

import numpy as np

K = 64
N = 500000
NCORES = 8
KPC = K // NCORES  # rows (replicates) per core
P = 128  # SBUF partitions
PPR = P // KPC  # partitions per row = 16
W = (KPC * N) // P  # free-dim elements per partition = 31250
SCALE = float((1.0 - 0.8**100) / N)

# Tunables (see bench.py for the A/B history). Best measured: "hw" impl —
# all bulk loads HWDGE alternating the two HW rings (sync SP + scalar ACT),
# geometric-taper tail chunks, no wait on the out-store receipt.
# Fast-path ~51.5-52 us/core (vs ~53 SWDGE raw, ~62 prior default); a
# run-varying subset of (mostly even-id) cores lands at ~56-61 us when one
# SDMA engine (15 or 0) runs ~20% degraded for the whole run — present in
# every DGE configuration tried; HWDGE shrinks its magnitude vs SWDGE.
CFG = dict(
    tile_w=1250,  # (128, 1250) f32 = 640 KB per DMA
    tail_split="taper",  # halve the trailing chunk repeatedly: tiny last reduce
    hw_head=0,  # (raw impl only)
    no_gpsimd_drain=True,  # all SWDGE DMAs are sem-waited; skip 2.5us dge_drain
    drop_const_memsets=True,  # dead framework memsets anchor the profile window
    act_share=2,  # alternate chunk reductions DVE/ACT: halves reduce critical path
    impl="hw",  # "hw" (HWDGE bulk) | "raw" (SWDGE, manual sems) | "tile"
    rings=2,  # stripe bulk loads across both HWDGE rings (SP + ACT)
    wait_out=False,  # out-store receipt overlaps the NEFF sem-sweep epilogue
    n_queues=1,
    tail="matmul",
)

_CACHED_NC = None


def _build_raw(cfg=CFG):
    """Raw bacc kernel: manual semaphores, no TileContext. Avoids Tile's
    kernel-tail double-barrier + per-sem reset storm (~8 us) and the ACT
    table preamble (no ScalarE ops)."""
    from contextlib import ExitStack

    import concourse.bacc as bacc
    import concourse.mybir as mybir

    tile_w = cfg["tile_w"]
    nt = W // tile_w
    assert nt * tile_w == W
    # Split the last chunk finer to shrink the trailing-reduce latency
    # after the final DMA lands.
    tail_split = cfg.get("tail_split", 2)
    if tail_split == "taper":
        # Geometric taper: halve the trailing chunk repeatedly so the DVE
        # reduce remaining after the last byte lands is minimal.
        tail, rest = [], tile_w
        while rest > tile_w // 8:
            tail.append(rest // 2)
            rest -= rest // 2
        tail.append(rest)
        widths = [tile_w] * (nt - 1) + tail
    else:
        base, rem = divmod(tile_w, tail_split)
        widths = [tile_w] * (nt - 1) + [
            base + (1 if j < rem else 0) for j in range(tail_split)
        ]
    assert sum(widths) == W
    nchunks = len(widths)
    edges = [0]
    for w_ in widths:
        edges.append(edges[-1] + w_)

    swdge_queues = cfg.get("swdge_queues", 1)
    nc = bacc.Bacc(
        "TRN2",
        target_bir_lowering=False,
        dynamic_dma_scratch_size=cfg.get("dma_scratch", 16384),
        num_swdge_queues=swdge_queues,
    )
    x = nc.dram_tensor("x", [P, W], mybir.dt.float32, kind="ExternalInput")
    sel = nc.dram_tensor("sel", [P, KPC], mybir.dt.float32, kind="ExternalInput")
    out = nc.dram_tensor("out", [KPC], mybir.dt.float32, kind="ExternalOutput")

    with ExitStack() as ctx:
        tiles = [
            ctx.enter_context(
                nc.sbuf_tensor(f"tile{i}", [P, widths[i]], mybir.dt.float32)
            )
            for i in range(nchunks)
        ]
        sel_t = ctx.enter_context(nc.sbuf_tensor([P, KPC], mybir.dt.float32))
        partials = ctx.enter_context(nc.sbuf_tensor([P, nchunks], mybir.dt.float32))
        res = ctx.enter_context(nc.sbuf_tensor([KPC, 1], mybir.dt.float32))
        acc = ctx.enter_context(nc.psum_tensor([KPC, nchunks], mybir.dt.float32))
        # One sem per DMA: a DMA's 16 lane-final descriptors each inc by 1,
        # so a shared running sem can hit 16*(i+1) with lane skew before
        # tile i fully lands. Dedicated sems waited to >=16 are exact.
        tile_sems = [
            ctx.enter_context(nc.semaphore(f"tsem{i}")) for i in range(nchunks)
        ]
        sel_sem = ctx.enter_context(nc.semaphore())
        out_sem = ctx.enter_context(nc.semaphore())
        vec_sem = ctx.enter_context(nc.semaphore())
        pe_sem = ctx.enter_context(nc.semaphore())
        res_sem = ctx.enter_context(nc.semaphore())
        # Every SWDGE DMA's completion is sem-waited by a consumer before the
        # block ends, so GpSimd's ~2.5us dge_drain at block exit is redundant.
        block = ctx.enter_context(
            nc.Block(no_gpsimd_drain=cfg.get("no_gpsimd_drain", False))
        )

        hw_head = cfg.get("hw_head", 0)  # leading chunks issued on HWDGE (hurts; keep 0)

        @block.gpsimd
        def _(g):
            for i in range(hw_head, nchunks):
                d = g.dma_start(
                    out=tiles[i][:], in_=x[:, edges[i] : edges[i + 1]]
                ).then_inc(tile_sems[i], 16)
                if swdge_queues > 1 and i % swdge_queues:
                    d.ins.queue = f"qPoolDynamic{i % swdge_queues}"

        # Split chunk reductions between DVE (reduce_sum) and ACT
        # (activation Copy with accum_out): halves the reduce-side critical
        # path so compute never falls behind the DMA stream.
        act_share = cfg.get("act_share", 2)  # every act_share-th chunk -> ACT
        # ACT takes alternate chunks, but NOT the final one: ACT's two-op
        # chain (Copy + accum write) is slower than DVE's single reduce, so
        # the last-landing chunk goes to DVE (swap the tail pair's parity).
        act_chunks = (
            [i for i in range(nchunks) if (i % act_share == 1) != (i >= nchunks - 2)]
            if act_share
            else []
        )
        dve_chunks = [i for i in range(nchunks) if i not in act_chunks]
        act_sem = ctx.enter_context(nc.semaphore())
        if act_chunks:
            act_scratch = ctx.enter_context(
                nc.sbuf_tensor([P, max(widths)], mybir.dt.float32)
            )

        @block.scalar
        def _(sc):
            a = None
            for i in act_chunks:
                sc.wait_ge(tile_sems[i], 16)
                a = sc.activation(
                    out=act_scratch[:, : widths[i]],
                    in_=tiles[i][:],
                    func=mybir.ActivationFunctionType.Copy,
                    accum_out=partials[:, i : i + 1],
                )
            if a is not None:
                a.then_inc(act_sem, 1)

        @block.vector
        def _(v):
            for i in dve_chunks:
                v.wait_ge(tile_sems[i], 16)
                r = v.reduce_sum(
                    out=partials[:, i : i + 1],
                    in_=tiles[i][:],
                    axis=mybir.AxisListType.X,
                )
            r.then_inc(vec_sem, 1)
            # Reduce the matmul's (KPC, nchunks) group-sums to (KPC, 1).
            v.wait_ge(pe_sem, 1)
            v.reduce_sum(
                out=res[:], in_=acc[:], axis=mybir.AxisListType.X
            ).then_inc(res_sem, 1)

        @block.tensor
        def _(t):
            # acc[m, c] = sum_p sel[p, m] * partials[p, c] (scale folded in sel)
            t.wait_ge(sel_sem, 16)
            t.wait_ge(vec_sem, 1)
            if act_chunks:
                t.wait_ge(act_sem, 1)
            nc.tensor.matmul(
                acc[:], sel_t[:], partials[:], start=True, stop=True
            ).then_inc(pe_sem, 1)

        @block.sync
        def _(s):
            # HWDGE leads: first bytes flow before the Q7 SWDGE wakes up.
            for i in range(hw_head):
                s.dma_start(
                    out=tiles[i][:], in_=x[:, edges[i] : edges[i + 1]]
                ).then_inc(tile_sems[i], 16)
            s.dma_start(out=sel_t[:], in_=sel[:, :]).then_inc(sel_sem, 16)
            # HWDGE out-store: no Q7 wake/emission on the critical tail.
            s.wait_ge(res_sem, 1)
            s.dma_start(out=out[:], in_=res[:, 0]).then_inc(out_sem, 16)
            if cfg.get("wait_out", True):
                # The SP Drain at block exit also flushes the HWDGE FIFO;
                # this explicit wait keeps the write-receipt on the critical
                # path (safe default).
                s.wait_ge(out_sem, 16)

    if cfg.get("drop_const_memsets", False):
        # The framework's 4 const-tile memsets ([128,1] each) have no readers
        # in this kernel; walrus flags them dead. They anchor gauge's
        # first_useful_time ~3 us before our first DMA packet.
        main = nc.m.functions[0].blocks[0]
        dead = [
            i
            for i in main.instructions
            if type(i).__name__ == "InstMemset"
            and any("const-" in str(o) for o in i.outs)
        ]
        for i in dead:
            main.instructions.remove(i)

    nc.compile()
    return nc


def _build_hw(cfg=CFG):
    """All bulk loads on HWDGE (sync SP ring; optionally alternating with the
    scalar/ACT ring). No Q7 descriptor emission on the bulk path — dodges both
    the SWDGE emission rate and the SDMA-7/15 descriptor-ring contention."""
    from contextlib import ExitStack

    import concourse.bacc as bacc
    import concourse.mybir as mybir

    tile_w = cfg["tile_w"]
    nt = W // tile_w
    assert nt * tile_w == W
    tail_split = cfg.get("tail_split", 2)
    if tail_split == "taper":
        # Geometric taper on the last chunk: the reduce remaining after the
        # final byte lands is ~tile_w/8 columns instead of tile_w/2.
        tail, rest = [], tile_w
        while rest > tile_w // 8:
            tail.append(rest // 2)
            rest -= rest // 2
        tail.append(rest)
        widths = [tile_w] * (nt - 1) + tail
    else:
        base, rem = divmod(tile_w, tail_split)
        widths = [tile_w] * (nt - 1) + [
            base + (1 if j < rem else 0) for j in range(tail_split)
        ]
    assert sum(widths) == W
    nchunks = len(widths)
    edges = [0]
    for w_ in widths:
        edges.append(edges[-1] + w_)

    rings = cfg.get("rings", 1)

    nc = bacc.Bacc(
        "TRN2",
        target_bir_lowering=False,
        dynamic_dma_scratch_size=cfg.get("dma_scratch", 16384),
    )
    x = nc.dram_tensor("x", [P, W], mybir.dt.float32, kind="ExternalInput")
    sel = nc.dram_tensor("sel", [P, KPC], mybir.dt.float32, kind="ExternalInput")
    out = nc.dram_tensor("out", [KPC], mybir.dt.float32, kind="ExternalOutput")

    with ExitStack() as ctx:
        tiles = [
            ctx.enter_context(
                nc.sbuf_tensor(f"tile{i}", [P, widths[i]], mybir.dt.float32)
            )
            for i in range(nchunks)
        ]
        sel_t = ctx.enter_context(nc.sbuf_tensor([P, KPC], mybir.dt.float32))
        partials = ctx.enter_context(nc.sbuf_tensor([P, nchunks], mybir.dt.float32))
        res = ctx.enter_context(nc.sbuf_tensor([KPC, 1], mybir.dt.float32))
        acc = ctx.enter_context(nc.psum_tensor([KPC, nchunks], mybir.dt.float32))
        tile_sems = [
            ctx.enter_context(nc.semaphore(f"tsem{i}")) for i in range(nchunks)
        ]
        sel_sem = ctx.enter_context(nc.semaphore())
        out_sem = ctx.enter_context(nc.semaphore())
        vec_sem = ctx.enter_context(nc.semaphore())
        pe_sem = ctx.enter_context(nc.semaphore())
        res_sem = ctx.enter_context(nc.semaphore())
        act_sem = ctx.enter_context(nc.semaphore())
        block = ctx.enter_context(
            nc.Block(no_gpsimd_drain=cfg.get("no_gpsimd_drain", False))
        )

        # sel rides the otherwise-idle SWDGE ring: zero interference with the
        # HWDGE bulk stream.
        @block.gpsimd
        def _(g):
            g.dma_start(out=sel_t[:], in_=sel[:, :]).then_inc(sel_sem, 16)

        act_share = cfg.get("act_share", 2)
        act_chunks = (
            [i for i in range(nchunks) if (i % act_share == 1) != (i >= nchunks - 2)]
            if act_share
            else []
        )
        dve_chunks = [i for i in range(nchunks) if i not in act_chunks]
        if act_chunks:
            act_scratch = ctx.enter_context(
                nc.sbuf_tensor([P, max(widths)], mybir.dt.float32)
            )

        # Chunk -> issuing ring. mix="alt": alternate the SWDGE (gpsimd) ring
        # with HWDGE so neither path's descriptor-fetch port takes the full
        # per-engine descriptor load (SWDGE pressures SDMA 7/15; HWDGE
        # pressures SDMA 0).
        mix = cfg.get("mix", None)
        gp_dma_chunks = []
        if mix == "alt":
            gp_dma_chunks = [i for i in range(nchunks) if i % 2 == 1]
            rest = [i for i in range(nchunks) if i % 2 == 0]
        else:
            rest = list(range(nchunks))
        sync_dma_chunks = [i for i in rest if rings == 1 or i % 2 == 0]
        scalar_dma_chunks = [i for i in rest if rings > 1 and i % 2 == 1]

        if gp_dma_chunks:
            @block.gpsimd
            def _(g):
                for i in gp_dma_chunks:
                    g.dma_start(
                        out=tiles[i][:], in_=x[:, edges[i] : edges[i + 1]]
                    ).then_inc(tile_sems[i], 16)

        @block.scalar
        def _(sc):
            # Issue this ring's share of bulk loads FIRST (issue is cheap);
            # only then start chewing on reduces, so later DMAs aren't
            # stuck behind compute waits in the sequencer.
            for i in scalar_dma_chunks:
                sc.dma_start(
                    out=tiles[i][:], in_=x[:, edges[i] : edges[i + 1]]
                ).then_inc(tile_sems[i], 16)
            a = None
            for i in act_chunks:
                sc.wait_ge(tile_sems[i], 16)
                a = sc.activation(
                    out=act_scratch[:, : widths[i]],
                    in_=tiles[i][:],
                    func=mybir.ActivationFunctionType.Copy,
                    accum_out=partials[:, i : i + 1],
                )
            if a is not None:
                a.then_inc(act_sem, 1)

        @block.vector
        def _(v):
            for i in dve_chunks:
                v.wait_ge(tile_sems[i], 16)
                r = v.reduce_sum(
                    out=partials[:, i : i + 1],
                    in_=tiles[i][:],
                    axis=mybir.AxisListType.X,
                )
            r.then_inc(vec_sem, 1)
            v.wait_ge(pe_sem, 1)
            v.reduce_sum(
                out=res[:], in_=acc[:], axis=mybir.AxisListType.X
            ).then_inc(res_sem, 1)

        @block.tensor
        def _(t):
            t.wait_ge(sel_sem, 16)
            t.wait_ge(vec_sem, 1)
            if act_chunks:
                t.wait_ge(act_sem, 1)
            nc.tensor.matmul(
                acc[:], sel_t[:], partials[:], start=True, stop=True
            ).then_inc(pe_sem, 1)

        @block.sync
        def _(s):
            for i in sync_dma_chunks:
                s.dma_start(
                    out=tiles[i][:], in_=x[:, edges[i] : edges[i + 1]]
                ).then_inc(tile_sems[i], 16)
            s.wait_ge(res_sem, 1)
            s.dma_start(out=out[:], in_=res[:, 0]).then_inc(out_sem, 16)
            if cfg.get("wait_out", True):
                s.wait_ge(out_sem, 16)

    if cfg.get("drop_const_memsets", False):
        main = nc.m.functions[0].blocks[0]
        dead = [
            i
            for i in main.instructions
            if type(i).__name__ == "InstMemset"
            and any("const-" in str(o) for o in i.outs)
        ]
        for i in dead:
            main.instructions.remove(i)

    nc.compile()
    return nc


def _build_bass(cfg=CFG):
    import concourse.bacc as bacc
    import concourse.mybir as mybir
    import concourse.tile as tile

    if cfg.get("impl", "tile") == "raw":
        return _build_raw(cfg)
    if cfg.get("impl", "tile") == "hw":
        return _build_hw(cfg)

    tile_w = cfg["tile_w"]
    n_queues = cfg["n_queues"]
    tail_split = cfg["tail_split"]
    nt = W // tile_w
    assert nt * tile_w == W

    nc = bacc.Bacc(
        "TRN2",
        target_bir_lowering=False,
        dynamic_dma_scratch_size=cfg.get("dma_scratch", 16384),
    )
    x = nc.dram_tensor("x", [P, W], mybir.dt.float32, kind="ExternalInput")
    if cfg["tail"] == "matmul":
        sel = nc.dram_tensor("sel", [P, KPC], mybir.dt.float32, kind="ExternalInput")
    out = nc.dram_tensor("out", [KPC], mybir.dt.float32, kind="ExternalOutput")
    if cfg["tail"] == "bounce":
        tmp = nc.dram_tensor("tmp", [P], mybir.dt.float32)

    # Chunk boundaries: full tiles except the last, which is split finer so
    # the trailing reduce latency after the final DMA is small.
    edges = [i * tile_w for i in range(nt)]
    last = edges.pop()
    step = tile_w // tail_split
    edges += [last + j * step for j in range(tail_split)]
    edges.append(W)
    n_chunks = len(edges) - 1

    with tile.TileContext(nc) as tc:
        with (
            tc.tile_pool(name="data", bufs=n_chunks) as data_pool,
            tc.tile_pool(name="small", bufs=1) as small,
        ):
            if cfg["tail"] == "matmul":
                sel_t = small.tile([P, KPC], mybir.dt.float32)
                nc.gpsimd.dma_start(out=sel_t, in_=sel[:, :])

            # Independent DMA rings: SWDGE (gpsimd) + the two HWDGE rings
            # (sync=SP, scalar=ACT). Striping loads across them keeps the
            # SDMA engines fed even when one ring hiccups.
            engines = [nc.gpsimd, nc.sync, nc.scalar][: max(1, min(n_queues, 3))]
            partials = small.tile([P, n_chunks], mybir.dt.float32)
            for i in range(n_chunks):
                lo, hi = edges[i], edges[i + 1]
                t = data_pool.tile([P, hi - lo], mybir.dt.float32, tag="data")
                engines[i % len(engines)].dma_start(out=t, in_=x[:, lo:hi])
                nc.vector.reduce_sum(
                    out=partials[:, i : i + 1], in_=t, axis=mybir.AxisListType.X
                )

            colsum = small.tile([P, 1], mybir.dt.float32)
            nc.vector.reduce_sum(out=colsum, in_=partials, axis=mybir.AxisListType.X)

            if cfg["tail"] == "matmul":
                # sel carries the 1/N * (1-0.8^100) scale, so the matmul
                # output is final; DVE copies PSUM->SBUF (DMA can't read PSUM).
                with tc.tile_pool(name="psum", bufs=1, space="PSUM") as psum_pool:
                    acc = psum_pool.tile([KPC, 1], mybir.dt.float32)
                    nc.tensor.matmul(acc, sel_t, colsum, start=True, stop=True)
                    res = small.tile([KPC, 1], mybir.dt.float32)
                    nc.vector.tensor_copy(res, acc)
                    nc.gpsimd.dma_start(out=out[:], in_=res[:, 0])
            else:
                nc.gpsimd.dma_start(out=tmp[:], in_=colsum[:, 0])
                row = small.tile([1, P], mybir.dt.float32)
                nc.gpsimd.dma_start(out=row, in_=tmp[None, :])
                rowsums = small.tile([1, KPC], mybir.dt.float32)
                nc.vector.reduce_sum(
                    out=rowsums,
                    in_=row.rearrange("p (k g) -> p k g", g=PPR),
                    axis=mybir.AxisListType.X,
                )
                res = small.tile([1, KPC], mybir.dt.float32)
                nc.scalar.mul(out=res, in_=rowsums, mul=SCALE)
                nc.gpsimd.dma_start(out=out[:], in_=res[0, :])

    nc.compile()
    return nc


def _get_nc():
    global _CACHED_NC
    if _CACHED_NC is None:
        _CACHED_NC = _build_bass()
    return _CACHED_NC


def _sel_matrix():
    sel = np.zeros((P, KPC), dtype=np.float32)
    sel[np.arange(P), np.arange(P) // PPR] = np.float32(SCALE)
    return sel


def _make_in_maps(replicates: np.ndarray, cfg=CFG):
    sel = _sel_matrix()
    in_maps = []
    for c in range(NCORES):
        shard = np.ascontiguousarray(
            replicates[c * KPC : (c + 1) * KPC].reshape(P, W)
        )
        m = {"x": shard}
        if cfg.get("tail", "matmul") == "matmul":
            m["sel"] = sel
        in_maps.append(m)
    return in_maps


def kernel(replicates: np.ndarray) -> np.ndarray:
    from concourse.bass_utils import run_bass_kernel_spmd

    assert replicates.shape == (K, N) and replicates.dtype == np.float32
    nc = _get_nc()
    res = run_bass_kernel_spmd(nc, _make_in_maps(replicates), list(range(NCORES)))
    return np.concatenate(
        [res.results[c]["out"].reshape(KPC) for c in range(NCORES)]
    ).astype(np.float32)

